# revision 46
# baseline (speedup 1.0000x reference)
"""PiCANet-G attention module as a Trainium2 Bass/Tile kernel.

Pure data-parallel over batch: 64 samples -> 8 cores x 8 samples.

Per core, three phases (all SBUF-resident, fp8 DoubleRow LSTM matmuls,
fp16 elementwise, fp16 fc/einsum, fp32 PSUM accumulation):
  P1: vertical bi-LSTM over W (batch = 8*28 (b, r) rows, 28 steps, 2 dirs)
  P2: horizontal bi-LSTM over H (batch = 8*28 (b, w) rows)
  P3: fc -> softmax(100) -> per-sample einsum with the dilated 10x10 patch

Layouts:
  - Gates in PSUM as [i,f,o] (3 banks) + [g] (1 bank), halves at 0/256;
    both tags double-buffered -> exactly 8 banks.
  - LSTM matmuls are fp8e4m3 DoubleRow (2 k-tiles per instruction).
  - Cell state c, gate activations, and h are fp16 in SBUF so DVE runs
    in its fast 2-byte mode; h is also materialized in fp8 for the next
    matmul (ring buffer per direction + full slab for the next phase).
  - Error (vs fp32 reference) measured at ~7e-3 relmax in a bit-accurate
    numpy model of this precision scheme.
"""

import numpy as np
import ml_dtypes
from contextlib import ExitStack

import concourse.bacc as bacc
import concourse.mybir as mybir
import concourse.tile as tile
from concourse.masks import make_identity
from concourse.bass_utils import run_bass_kernel_spmd

# problem shapes (hardcoded per contract)
B, C, H, W = 64, 512, 28, 28
HID = 256
N_CORES = 8
BL = B // N_CORES        # samples per core
NB = BL * H              # 224 rows per LSTM step
T = 28                   # steps per LSTM
PLOC = BL * H * W        # 6272 positions per core

F8 = mybir.dt.float8e4
F16 = mybir.dt.float16
BF16 = mybir.dt.bfloat16
F32 = mybir.dt.float32
AF = mybir.ActivationFunctionType
DR = mybir.MatmulPerfMode.DoubleRow

# torch gate order [i f g o] -> device order [i f o g] (sigmoids first)
_PERM = np.concatenate([np.arange(0, 512), np.arange(768, 1024), np.arange(512, 768)])
_LSTMS = ["vf", "vb", "hf", "hb"]


def _emit_pair(nc, gp, scr, wihs, whhs, biasTs, ones224, rhs_wih, rings, cs,
               t, phase, sfx, has_bias, h16_dst=None, hv_dst=None):
    """One LSTM step for both directions, ops grouped by engine.

    rhs_wih(d, j, pos) -> fp8 [128, 2, 224] AP for the input projection.
    rings[d]: fp8 [128, 3, 2, 224] ring (slot t%3 written, (t-1)%3 read).
    cs[d]: fp16 [128, 2, 224] cell state (in-place).
    h16_dst(d, pos): fp16 [128, 2, 224] AP (P2 only, feeds the fc).
    hv_dst(d, pos): strided fp8 AP for the next-phase slab (P1 only;
        written by the Pool engine off the critical path).
    """
    poss = (t, T - 1 - t)
    dirs = (0, 1)
    gifo, gg = {}, {}
    for d in range(2):
        gifo[d] = gp.tile([128, 3, 512], F32, tag="ifo", name=f"ifo_{sfx}_{t}_{d}")
        gg[d] = gp.tile([128, 512], F32, tag="gg", name=f"gg_{sfx}_{t}_{d}")

    def out_ap(d, gate, q):
        if gate < 3:
            return gifo[d][:, gate, q * 256: q * 256 + 224]
        return gg[d][:, q * 256: q * 256 + 224]

    # --- PE: gate projections. Each PSUM region's accumulation group
    # must be CONTIGUOUS on the PE queue (hw start/stop are group
    # delimiters, not per-region flags), so Wih+Whh+bias are emitted
    # back-to-back per (dir, gate, half). Region order [i, f, g, o]:
    # the [i,f] activation (the recurrence-critical one) can start
    # after only 6 of 8 groups.
    for d in dirs:
        pos = poss[d]
        prev_slot = (t - 1) % 3
        for gate in (0, 1, 3, 2):
            for q in range(2):
                m = gate * 2 + q
                for j in range(2):
                    nc.tensor.matmul(
                        out_ap(d, gate, q),
                        lhsT=wihs[d][:, 2 * j:2 * j + 2, m * 128:(m + 1) * 128],
                        rhs=rhs_wih(d, j, pos),
                        start=(j == 0),
                        stop=(t == 0 and not has_bias and j == 1),
                        perf_mode=DR,
                    )
                if t > 0:
                    nc.tensor.matmul(
                        out_ap(d, gate, q),
                        lhsT=whhs[d][:, 0:2, m * 128:(m + 1) * 128],
                        rhs=rings[d][:, prev_slot],
                        start=False,
                        stop=not has_bias,
                        perf_mode=DR,
                    )
                if has_bias:
                    nc.tensor.matmul(
                        out_ap(d, gate, q),
                        lhsT=biasTs[d][0:1, m * 128:(m + 1) * 128],
                        rhs=ones224,
                        start=False,
                        stop=True,
                    )

    # --- Act: i/f sigmoids + g tanh first (cell-update critical path) ---
    sif, so, tg, th = {}, {}, {}, {}
    for d in dirs:
        sif[d] = scr.tile([128, 2, 2, 224], F16, tag="sif", bufs=4,
                          name=f"sif_{sfx}_{t}_{d}")
        gv = gifo[d].rearrange("p g (h x) -> p g h x", h=2)[:, 0:2, :, 0:224]
        nc.scalar.activation(sif[d], gv, AF.Sigmoid)
        tg[d] = scr.tile([128, 2, 224], F16, tag="tg", bufs=4,
                         name=f"tg_{sfx}_{t}_{d}")
        ggv = gg[d].rearrange("p (h x) -> p h x", h=2)[:, :, 0:224]
        nc.scalar.activation(tg[d], ggv, AF.Tanh)

    # --- DVE: cell update (all-fp16 SBUF = fast mode) ---
    for d in dirs:
        if t == 0:
            nc.vector.tensor_mul(cs[d], sif[d][:, 0], tg[d])
        else:
            t1 = scr.tile([128, 2, 224], F16, tag="t1", bufs=4,
                          name=f"t1_{sfx}_{t}_{d}")
            nc.vector.tensor_mul(t1, sif[d][:, 0], tg[d])
            nc.vector.tensor_mul(cs[d], sif[d][:, 1], cs[d])
            nc.vector.tensor_add(cs[d], cs[d], t1)

    # --- Act: tanh(c) then o sigmoid ---
    for d in dirs:
        th[d] = scr.tile([128, 2, 224], F16, tag="th", bufs=4,
                         name=f"th_{sfx}_{t}_{d}")
        nc.scalar.activation(th[d], cs[d], AF.Tanh)
        so[d] = scr.tile([128, 2, 224], F16, tag="so", bufs=4,
                         name=f"so_{sfx}_{t}_{d}")
        ov = gifo[d].rearrange("p g (h x) -> p g h x", h=2)[:, 2, :, 0:224]
        nc.scalar.activation(so[d], ov, AF.Sigmoid)

    # --- DVE: h outputs (fp8 ring; fp16 slab for P2). The last step's
    # ring slot has no reader, so P1's final h goes straight to the
    # slab (skipping the Pool scatter on the phase boundary) and P2's
    # final ring write is dropped ---
    last = (t == T - 1)
    for d in dirs:
        slot = t % 3
        if not last:
            nc.vector.tensor_mul(rings[d][:, slot], so[d], th[d])
        elif hv_dst is not None:
            nc.vector.tensor_mul(
                hv_dst(d, poss[d]),
                so[d].rearrange("p q (b r) -> p q b r", b=BL),
                th[d].rearrange("p q (b r) -> p q b r", b=BL))
        if h16_dst is not None:
            nc.vector.tensor_mul(h16_dst(d, poss[d]), so[d], th[d])

    # --- Pool: scatter h into the next phase's slab (P1 only) ---
    if hv_dst is not None and not last:
        for d in dirs:
            slot = t % 3
            src = rings[d][:, slot].rearrange("p q (b r) -> p q b r", b=BL)
            nc.gpsimd.tensor_copy(hv_dst(d, poss[d]), src)


def _build(reps=1, debug=False, has_bias=True):
    nc = bacc.Bacc(None, target_bir_lowering=False)

    xT_d = nc.dram_tensor("xT", [C, PLOC], F8, kind="ExternalInput")
    w_d = {}
    for L in _LSTMS:
        din = C if L in ("vf", "vb") else 2 * HID
        w_d[L + "_wih"] = nc.dram_tensor(L + "_wih", [din, 1024], F8, kind="ExternalInput")
        w_d[L + "_whh"] = nc.dram_tensor(L + "_whh", [256, 1024], F8, kind="ExternalInput")
        w_d[L + "_bias"] = nc.dram_tensor(L + "_bias", [1, 1024], BF16, kind="ExternalInput")
    fcw_d = nc.dram_tensor("fcw", [512, 100], F16, kind="ExternalInput")
    fcb_d = nc.dram_tensor("fcb", [1, 100], F16, kind="ExternalInput")
    patchT_d = nc.dram_tensor("patchT", [BL, 100, 512], F16, kind="ExternalInput")
    out_d = nc.dram_tensor("out", [BL, C, H * W], F16, kind="ExternalOutput")
    if debug:
        dbg_hv = nc.dram_tensor("dbg_hv", [128, T, 4, 224], F8, kind="ExternalOutput")
        dbg_hh = nc.dram_tensor("dbg_hh", [128, 4, PLOC], F16, kind="ExternalOutput")
        dbg_kt = nc.dram_tensor("dbg_kt", [100, PLOC], F16, kind="ExternalOutput")

    with tile.TileContext(nc) as tc, ExitStack() as ctx:
        wpool = ctx.enter_context(tc.tile_pool(name="wpool", bufs=1))
        big = ctx.enter_context(tc.tile_pool(name="big", bufs=1))
        state = ctx.enter_context(tc.tile_pool(name="state", bufs=1))
        scr = ctx.enter_context(tc.tile_pool(name="scr", bufs=3))

        # --- load weights; stage-1 dirs first (step 0 needs them) ---
        wih_sb, whh_sb, biasT_sb = {}, {}, {}
        for L in _LSTMS:
            wih_sb[L] = wpool.tile([128, 4, 1024], F8, name=f"wih_{L}")
            whh_sb[L] = wpool.tile([128, 2, 1024], F8, name=f"whh_{L}")
            if has_bias:
                biasT_sb[L] = wpool.tile([1, 1024], BF16, name=f"bias_{L}")
            else:
                biasT_sb[L] = None
        if has_bias:
            ones224 = wpool.tile([1, 224], BF16, name="ones224")
            nc.vector.memset(ones224, 1.0)
            ones112 = wpool.tile([1, 112], F16, name="ones112")
            nc.vector.memset(ones112, 1.0)
        else:
            ones224 = ones112 = None
        fcw_sb = wpool.tile([128, 4, 100], F16, name="fcw_sb")
        if has_bias:
            fcb_sb = wpool.tile([1, 100], F16, name="fcb_sb")
        else:
            fcb_sb = None
        patchT_sb = wpool.tile([100, BL, 512], F16, name="patchT_sb")
        ident = wpool.tile([112, 112], F16, name="ident")
        make_identity(nc, ident)

        # round-robin input loads over sync+gpsimd only, in first-use
        # order (step-0 x blocks + vf/vb weights first). The scalar
        # queue is the Act engine's: a DMA issue there costs ~1.3us of
        # Act-sequencer time and would stall the first activations.
        _dq = [nc.sync, nc.gpsimd]
        _di = [0]

        def load(dst, src):
            _dq[_di[0] % 2].dma_start(out=dst, in_=src)
            _di[0] += 1

        def load_wih(L):
            src = w_d[L + "_wih"].rearrange("(kt p) m -> p kt m", kt=4)
            for kk in range(4):
                load(wih_sb[L][:, kk], src[:, kk])

        def load_whh(L):
            src = w_d[L + "_whh"].rearrange("(kt p) m -> p kt m", kt=2)
            for kk in range(2):
                load(whh_sb[L][:, kk], src[:, kk])
            if has_bias:
                load(biasT_sb[L], w_d[L + "_bias"][:, :])

        for rep in range(reps):
            sfx = f"r{rep}"
            # --- P1 input: x stream, edges first (both dirs consume edges);
            # on the first rep the weight/constant loads slot in between ---
            xT = big.tile([128, 4, PLOC], F8, tag="xT", name=f"xT_{sfx}")
            xsrc = xT_d.rearrange("(kt p) f -> p kt f", kt=4)
            wblocks = [(0, 3), (25, 28), (3, 8), (20, 25), (8, 14), (14, 20)]
            for bi, (lo, hi) in enumerate(wblocks):
                for kk in range(4):
                    load(xT[:, kk, lo * 224:hi * 224],
                         xsrc[:, kk, lo * 224:hi * 224])
                if rep == 0:
                    if bi == 1:
                        for L in ("vf", "vb"):
                            load_wih(L)
                            load_whh(L)
                    elif bi == 3:
                        for L in ("hf", "hb"):
                            load_wih(L)
                            load_whh(L)
                    elif bi == 5:
                        load(fcw_sb, fcw_d.rearrange("(kt p) n -> p kt n", kt=4))
                        if has_bias:
                            load(fcb_sb, fcb_d[:, :])
                        load(patchT_sb, patchT_d.rearrange("b k c -> k b c"))

            # P1 output slab, P2-read order: free = r*896 + kt*224 + (b*28+w)
            Hv = big.tile([128, T, 4, 224], F8, tag="Hv", name=f"Hv_{sfx}")
            HvS = Hv.rearrange("p r q (b w) -> p q b r w", b=BL)

            def rhs1(d, j, pos, _xT=xT):
                return _xT[:, 2 * j:2 * j + 2, pos * 224:(pos + 1) * 224]

            def hv_dst(d, pos, _HvS=HvS):
                return _HvS[:, 2 * d:2 * d + 2, :, :, pos]

            rings1 = [state.tile([128, 3, 2, 224], F8, tag=f"ring1_{d}",
                                 name=f"ring1_{d}_{sfx}") for d in range(2)]
            cs1 = [state.tile([128, 2, 224], F16, tag=f"c1_{d}",
                              name=f"c1_{d}_{sfx}") for d in range(2)]
            with tc.tile_pool(name="gates1", bufs=2, space="PSUM") as gp:
                for t in range(T):
                    _emit_pair(nc, gp, scr,
                               [wih_sb["vf"], wih_sb["vb"]],
                               [whh_sb["vf"], whh_sb["vb"]],
                               [biasT_sb["vf"], biasT_sb["vb"]], ones224,
                               rhs1, rings1, cs1, t, 1, f"1{sfx}", has_bias,
                               h16_dst=None, hv_dst=hv_dst)

            # --- P2: horizontal bi-LSTM ---
            Hh = big.tile([128, 4, PLOC], F16, tag="Hh", name=f"Hh_{sfx}")

            def rhs2(d, j, pos, _Hv=Hv):
                return _Hv[:, pos, 2 * j:2 * j + 2, :]

            def h16_dst(d, pos, _Hh=Hh):
                return _Hh[:, 2 * d:2 * d + 2, pos * 224:(pos + 1) * 224]

            rings2 = [state.tile([128, 3, 2, 224], F8, tag=f"ring2_{d}",
                                 name=f"ring2_{d}_{sfx}") for d in range(2)]
            cs2 = [state.tile([128, 2, 224], F16, tag=f"c2_{d}",
                              name=f"c2_{d}_{sfx}") for d in range(2)]
            with tc.tile_pool(name="gates2", bufs=2, space="PSUM") as gp:
                for t in range(T):
                    _emit_pair(nc, gp, scr,
                               [wih_sb["hf"], wih_sb["hb"]],
                               [whh_sb["hf"], whh_sb["hb"]],
                               [biasT_sb["hf"], biasT_sb["hb"]], ones224,
                               rhs2, rings2, cs2, t, 2, f"2{sfx}", has_bias,
                               h16_dst=h16_dst, hv_dst=None)

            # --- P3: fc + softmax + transpose + einsum ---
            KT = big.tile([100, PLOC], F16, tag="KT", name=f"KT_{sfx}")
            # GPSIMD cannot touch PSUM; alternate the PSUM->SBUF copies
            # between DVE and Act (Act has slack once the LSTMs are done)
            cps = [nc.vector, nc.scalar]
            dmas = [nc.sync]
            ci = 0
            with tc.tile_pool(name="p3ps", bufs=2, space="PSUM") as pps:

                def fc_tile(half, hr):
                    nonlocal ci
                    off = hr * 224 + half * 112
                    Lp = pps.tile([112, 100], F32, tag="L", bufs=3,
                                  name=f"L_{hr}_{half}_{sfx}")
                    for kk in range(4):
                        nc.tensor.matmul(Lp, lhsT=Hh[:, kk, off:off + 112],
                                         rhs=fcw_sb[:, kk, :],
                                         start=(kk == 0),
                                         stop=(not has_bias and kk == 3))
                    if has_bias:
                        nc.tensor.matmul(Lp, lhsT=ones112, rhs=fcb_sb,
                                         start=False, stop=True)
                    E = scr.tile([112, 100], F16, tag="E", bufs=3,
                                 name=f"E_{hr}_{half}_{sfx}")
                    Zs = scr.tile([112, 1], F32, tag="Z", bufs=3,
                                  name=f"Z_{hr}_{half}_{sfx}")
                    # accum_out would add a 187ns accumulator-read on the
                    # Act engine; the free-dim sum on DVE keeps the whole
                    # normalize chain on one engine
                    nc.scalar.activation(E, Lp, AF.Exp)
                    nc.vector.tensor_reduce(Zs, E, mybir.AxisListType.X,
                                            mybir.AluOpType.add)
                    rz = scr.tile([112, 1], F32, tag="rz", bufs=3,
                                  name=f"rz_{hr}_{half}_{sfx}")
                    nc.vector.reciprocal(rz, Zs)
                    Ka = scr.tile([112, 100], F16, tag="Ka", bufs=3,
                                  name=f"Ka_{hr}_{half}_{sfx}")
                    nc.vector.tensor_scalar_mul(Ka, E, rz)
                    KTp = pps.tile([100, 112], F16, tag="KTp",
                                   name=f"KTp_{hr}_{half}_{sfx}")
                    nc.tensor.transpose(KTp, Ka, ident)
                    # KT columns p = b*784 + hr*28 + w for these positions
                    dst = KT.rearrange("k (b hw) -> k b hw", b=BL)[
                        :, half * 4:(half + 1) * 4, hr * 28:(hr + 1) * 28]
                    nc.vector.tensor_copy(dst, KTp)
                    ci += 1

                def einsum_unit(b_i, ct, j2):
                    nonlocal ci
                    lhsT = patchT_sb[:, b_i, ct * 128:(ct + 1) * 128]
                    Op = pps.tile([128, 512], F32, tag="O", bufs=3,
                                  name=f"O_{b_i}_{ct}_{j2}_{sfx}")
                    nc.tensor.matmul(
                        Op[:, 0:392], lhsT=lhsT,
                        rhs=KT[:, b_i * 784 + j2 * 392:
                               b_i * 784 + (j2 + 1) * 392],
                        start=True, stop=True)
                    ob = scr.tile([128, 392], F16, tag="ob", bufs=4,
                                  name=f"ob_{b_i}_{ct}_{j2}_{sfx}")
                    eng = cps[ci % 2]
                    if eng is nc.scalar:
                        eng.copy(ob, Op[:, 0:392])
                    else:
                        eng.tensor_copy(ob, Op[:, 0:392])
                    nc.sync.dma_start(
                        out=out_d[b_i, ct * 128:(ct + 1) * 128,
                                  j2 * 392:(j2 + 1) * 392],
                        in_=ob)
                    ci += 1

                # einsum unit (b, ct, j2) needs KT rows r<14 (j2=0) or
                # r>=14 (j2=1) of its half: ready after 14 fc tiles.
                # Interleave ready units between fc tiles to keep every
                # engine busy through the softmax chain.
                pending = []
                nfc = 0
                for half in range(2):
                    for hr in range(H):
                        fc_tile(half, hr)
                        nfc += 1
                        if nfc % 14 == 0:
                            j2 = (nfc // 14 + 1) % 2
                            pending += [(b_i, ct, j2)
                                        for b_i in range(((nfc - 1) // 28) * 4,
                                                         ((nfc - 1) // 28) * 4 + 4)
                                        for ct in range(4)]
                        for _ in range(2):
                            if pending:
                                einsum_unit(*pending.pop(0))
                for u in pending:
                    einsum_unit(*u)
            if debug and rep == reps - 1:
                nc.sync.dma_start(out=dbg_hv[:, :, :, :], in_=Hv)
                nc.sync.dma_start(out=dbg_hh[:, :, :], in_=Hh)
                nc.sync.dma_start(out=dbg_kt[:, :], in_=KT)

    nc.compile()
    return nc


_NC_CACHE = {}


def _get_nc(reps=1, debug=False, has_bias=True):
    key = (reps, debug, has_bias)
    if key not in _NC_CACHE:
        _NC_CACHE[key] = _build(reps=reps, debug=debug, has_bias=has_bias)
    return _NC_CACHE[key]


def _prep_core_inputs(x, weights_np):
    """Host-side marshalling for one core. x: [BL, C, H, W] f32."""
    f8 = ml_dtypes.float8_e4m3
    m = {}
    m["xT"] = np.ascontiguousarray(
        x.transpose(1, 3, 0, 2).reshape(C, PLOC)).astype(f8)
    m["patchT"] = np.ascontiguousarray(
        x[:, :, ::3, ::3].reshape(BL, C, 100).transpose(0, 2, 1)).astype(np.float16)
    m.update(weights_np)
    return m


def _prep_weights(inputs):
    f8 = ml_dtypes.float8_e4m3
    bf = ml_dtypes.bfloat16
    w = {}
    for L in _LSTMS:
        wih = np.asarray(inputs[L + "_Wih"], np.float32)
        whh = np.asarray(inputs[L + "_Whh"], np.float32)
        bih = np.asarray(inputs[L + "_bih"], np.float32)
        bhh = np.asarray(inputs[L + "_bhh"], np.float32)
        w[L + "_wih"] = np.ascontiguousarray(wih[_PERM].T).astype(f8)
        w[L + "_whh"] = np.ascontiguousarray(whh[_PERM].T).astype(f8)
        w[L + "_bias"] = np.ascontiguousarray(
            (bih + bhh)[_PERM].reshape(1, 1024)).astype(bf)
    w["fcw"] = np.asarray(inputs["fc_W"], np.float32).astype(np.float16)
    w["fcb"] = np.asarray(inputs["fc_b"], np.float32).reshape(1, 100).astype(np.float16)
    return w


def run_cores(inputs, reps=1, debug=False):
    x = np.asarray(inputs["x"], np.float32)
    wnp = _prep_weights(inputs)
    has_bias = any(np.any(np.asarray(wnp[L + "_bias"], np.float32))
                   for L in _LSTMS) or np.any(np.asarray(wnp["fcb"], np.float32))
    nc = _get_nc(reps=reps, debug=debug, has_bias=has_bias)
    in_maps = [
        _prep_core_inputs(x[ci * BL:(ci + 1) * BL], wnp) for ci in range(N_CORES)
    ]
    res = run_bass_kernel_spmd(nc, in_maps, list(range(N_CORES)))
    return res


def kernel(**inputs) -> np.ndarray:
    res = run_cores(inputs)
    out = np.concatenate(
        [res.results[ci]["out"].reshape(BL, C, H, W) for ci in range(N_CORES)],
        axis=0)
    return out.astype(np.float32)


# revision 48
# speedup vs baseline: 3.3280x; 3.3280x over previous
"""PiCANet-G attention module as a Trainium2 Bass/Tile kernel.

Pure data-parallel over batch: 64 samples -> 8 cores x 8 samples.

Per core, three phases (all SBUF-resident, fp8 DoubleRow LSTM matmuls,
fp16 elementwise, fp16 fc/einsum, fp32 PSUM accumulation):
  P1: vertical bi-LSTM over W (batch = 8*28 (b, r) rows, 28 steps, 2 dirs)
  P2: horizontal bi-LSTM over H (batch = 8*28 (b, w) rows)
  P3: fc -> softmax(100) -> per-sample einsum with the dilated 10x10 patch

Layouts:
  - Gates in PSUM as [i,f,o] (3 banks) + [g] (1 bank), halves at 0/256;
    both tags double-buffered -> exactly 8 banks.
  - LSTM matmuls are fp8e4m3 DoubleRow (2 k-tiles per instruction).
  - Cell state c, gate activations, and h are fp16 in SBUF so DVE runs
    in its fast 2-byte mode; h is also materialized in fp8 for the next
    matmul (ring buffer per direction + full slab for the next phase).
  - Error (vs fp32 reference) measured at ~7e-3 relmax in a bit-accurate
    numpy model of this precision scheme.
"""

import numpy as np
import ml_dtypes
from contextlib import ExitStack

import concourse.bacc as bacc
import concourse.mybir as mybir
import concourse.tile as tile
from concourse.masks import make_identity
from concourse.bass_utils import run_bass_kernel_spmd

# problem shapes (hardcoded per contract)
B, C, H, W = 64, 512, 28, 28
HID = 256
N_CORES = 8
BL = B // N_CORES        # samples per core
NB = BL * H              # 224 rows per LSTM step
T = 28                   # steps per LSTM
PLOC = BL * H * W        # 6272 positions per core

F8 = mybir.dt.float8e4
F16 = mybir.dt.float16
BF16 = mybir.dt.bfloat16
F32 = mybir.dt.float32
AF = mybir.ActivationFunctionType
DR = mybir.MatmulPerfMode.DoubleRow

# torch gate order [i f g o] -> device order [i f o g] (sigmoids first)
_PERM = np.concatenate([np.arange(0, 512), np.arange(768, 1024), np.arange(512, 768)])
_LSTMS = ["vf", "vb", "hf", "hb"]


def _emit_pair(nc, gp, scr, wihs, whhs, biasTs, ones224, rhs_wih, rings, cs,
               t, phase, sfx, has_bias, h16_dst=None, hv_dst=None):
    """One LSTM step for both directions, ops grouped by engine.

    rhs_wih(d, j, pos) -> fp8 [128, 2, 224] AP for the input projection.
    rings[d]: fp8 [128, 3, 2, 224] ring (slot t%3 written, (t-1)%3 read).
    cs[d]: fp16 [128, 2, 224] cell state (in-place).
    h16_dst(d, pos): fp16 [128, 2, 224] AP (P2 only, feeds the fc).
    hv_dst(d, pos): strided fp8 AP for the next-phase slab (P1 only;
        written by the Pool engine off the critical path).
    """
    poss = (t, T - 1 - t)
    dirs = (0, 1)
    gifo, gg = {}, {}
    for d in range(2):
        gifo[d] = gp.tile([128, 3, 512], F32, tag="ifo", name=f"ifo_{sfx}_{t}_{d}")
        gg[d] = gp.tile([128, 512], F32, tag="gg", name=f"gg_{sfx}_{t}_{d}")

    def out_ap(d, gate, q):
        if gate < 3:
            return gifo[d][:, gate, q * 256: q * 256 + 224]
        return gg[d][:, q * 256: q * 256 + 224]

    # --- PE: gate projections. Each PSUM region's accumulation group
    # must be CONTIGUOUS on the PE queue (hw start/stop are group
    # delimiters, not per-region flags), so Wih+Whh+bias are emitted
    # back-to-back per (dir, gate, half). Region order [i, f, g, o]:
    # the [i,f] activation (the recurrence-critical one) can start
    # after only 6 of 8 groups.
    for d in dirs:
        pos = poss[d]
        prev_slot = (t - 1) % 3
        for gate in (0, 1, 3, 2):
            for q in range(2):
                m = gate * 2 + q
                for j in range(2):
                    nc.tensor.matmul(
                        out_ap(d, gate, q),
                        lhsT=wihs[d][:, 2 * j:2 * j + 2, m * 128:(m + 1) * 128],
                        rhs=rhs_wih(d, j, pos),
                        start=(j == 0),
                        stop=(t == 0 and not has_bias and j == 1),
                        perf_mode=DR,
                    )
                if t > 0:
                    nc.tensor.matmul(
                        out_ap(d, gate, q),
                        lhsT=whhs[d][:, 0:2, m * 128:(m + 1) * 128],
                        rhs=rings[d][:, prev_slot],
                        start=False,
                        stop=not has_bias,
                        perf_mode=DR,
                    )
                if has_bias:
                    nc.tensor.matmul(
                        out_ap(d, gate, q),
                        lhsT=biasTs[d][0:1, m * 128:(m + 1) * 128],
                        rhs=ones224,
                        start=False,
                        stop=True,
                    )

    # --- Act: i/f sigmoids + g tanh first (cell-update critical path) ---
    sif, so, tg, th = {}, {}, {}, {}
    for d in dirs:
        sif[d] = scr.tile([128, 2, 2, 224], F16, tag="sif", bufs=4,
                          name=f"sif_{sfx}_{t}_{d}")
        gv = gifo[d].rearrange("p g (h x) -> p g h x", h=2)[:, 0:2, :, 0:224]
        nc.scalar.activation(sif[d], gv, AF.Sigmoid)
        tg[d] = scr.tile([128, 2, 224], F16, tag="tg", bufs=4,
                         name=f"tg_{sfx}_{t}_{d}")
        ggv = gg[d].rearrange("p (h x) -> p h x", h=2)[:, :, 0:224]
        nc.scalar.activation(tg[d], ggv, AF.Tanh)

    # --- DVE: cell update (all-fp16 SBUF = fast mode) ---
    for d in dirs:
        if t == 0:
            nc.vector.tensor_mul(cs[d], sif[d][:, 0], tg[d])
        else:
            t1 = scr.tile([128, 2, 224], F16, tag="t1", bufs=4,
                          name=f"t1_{sfx}_{t}_{d}")
            nc.vector.tensor_mul(t1, sif[d][:, 0], tg[d])
            nc.vector.tensor_mul(cs[d], sif[d][:, 1], cs[d])
            nc.vector.tensor_add(cs[d], cs[d], t1)

    # --- Act: tanh(c) then o sigmoid ---
    for d in dirs:
        th[d] = scr.tile([128, 2, 224], F16, tag="th", bufs=4,
                         name=f"th_{sfx}_{t}_{d}")
        nc.scalar.activation(th[d], cs[d], AF.Tanh)
        so[d] = scr.tile([128, 2, 224], F16, tag="so", bufs=4,
                         name=f"so_{sfx}_{t}_{d}")
        ov = gifo[d].rearrange("p g (h x) -> p g h x", h=2)[:, 2, :, 0:224]
        nc.scalar.activation(so[d], ov, AF.Sigmoid)

    # --- DVE: h outputs (fp8 ring; fp16 slab for P2). The last step's
    # ring slot has no reader, so P1's final h goes straight to the
    # slab (skipping the Pool scatter on the phase boundary) and P2's
    # final ring write is dropped ---
    last = (t == T - 1)
    for d in dirs:
        slot = t % 3
        if not last:
            nc.vector.tensor_mul(rings[d][:, slot], so[d], th[d])
        elif hv_dst is not None:
            nc.vector.tensor_mul(
                hv_dst(d, poss[d]),
                so[d].rearrange("p q (b r) -> p q b r", b=BL),
                th[d].rearrange("p q (b r) -> p q b r", b=BL))
        if h16_dst is not None:
            nc.vector.tensor_mul(h16_dst(d, poss[d]), so[d], th[d])

    # --- Pool: scatter h into the next phase's slab (P1 only) ---
    if hv_dst is not None and not last:
        for d in dirs:
            slot = t % 3
            src = rings[d][:, slot].rearrange("p q (b r) -> p q b r", b=BL)
            nc.gpsimd.tensor_copy(hv_dst(d, poss[d]), src)


def _build(reps=1, debug=False, has_bias=True):
    nc = bacc.Bacc(None, target_bir_lowering=False)

    xT_d = nc.dram_tensor("xT", [C, PLOC], F8, kind="ExternalInput")
    w_d = {}
    for L in _LSTMS:
        din = C if L in ("vf", "vb") else 2 * HID
        w_d[L + "_wih"] = nc.dram_tensor(L + "_wih", [din, 1024], F8, kind="ExternalInput")
        w_d[L + "_whh"] = nc.dram_tensor(L + "_whh", [256, 1024], F8, kind="ExternalInput")
        w_d[L + "_bias"] = nc.dram_tensor(L + "_bias", [1, 1024], BF16, kind="ExternalInput")
    fcw_d = nc.dram_tensor("fcw", [512, 100], F16, kind="ExternalInput")
    fcb_d = nc.dram_tensor("fcb", [1, 100], F16, kind="ExternalInput")
    patchT_d = nc.dram_tensor("patchT", [BL, 100, 512], F16, kind="ExternalInput")
    out_d = nc.dram_tensor("out", [BL, C, H * W], F16, kind="ExternalOutput")
    if debug:
        dbg_hv = nc.dram_tensor("dbg_hv", [128, T, 4, 224], F8, kind="ExternalOutput")
        dbg_hh = nc.dram_tensor("dbg_hh", [128, 4, PLOC], F16, kind="ExternalOutput")
        dbg_kt = nc.dram_tensor("dbg_kt", [100, PLOC], F16, kind="ExternalOutput")

    with tile.TileContext(nc) as tc, ExitStack() as ctx:
        wpool = ctx.enter_context(tc.tile_pool(name="wpool", bufs=1))
        big = ctx.enter_context(tc.tile_pool(name="big", bufs=1))
        state = ctx.enter_context(tc.tile_pool(name="state", bufs=1))
        scr = ctx.enter_context(tc.tile_pool(name="scr", bufs=3))

        # --- load weights; stage-1 dirs first (step 0 needs them) ---
        wih_sb, whh_sb, biasT_sb = {}, {}, {}
        for L in _LSTMS:
            wih_sb[L] = wpool.tile([128, 4, 1024], F8, name=f"wih_{L}")
            whh_sb[L] = wpool.tile([128, 2, 1024], F8, name=f"whh_{L}")
            if has_bias:
                biasT_sb[L] = wpool.tile([1, 1024], BF16, name=f"bias_{L}")
            else:
                biasT_sb[L] = None
        if has_bias:
            ones224 = wpool.tile([1, 224], BF16, name="ones224")
            nc.vector.memset(ones224, 1.0)
            ones112 = wpool.tile([1, 112], F16, name="ones112")
            nc.vector.memset(ones112, 1.0)
        else:
            ones224 = ones112 = None
        fcw_sb = wpool.tile([128, 4, 100], F16, name="fcw_sb")
        if has_bias:
            fcb_sb = wpool.tile([1, 100], F16, name="fcb_sb")
        else:
            fcb_sb = None
        patchT_sb = wpool.tile([100, BL, 512], F16, name="patchT_sb")
        ident = wpool.tile([112, 112], F16, name="ident")
        make_identity(nc, ident)

        # round-robin input loads over sync+gpsimd only, in first-use
        # order (step-0 x blocks + vf/vb weights first). The scalar
        # queue is the Act engine's: a DMA issue there costs ~1.3us of
        # Act-sequencer time and would stall the first activations.
        _dq = [nc.sync, nc.gpsimd]
        _di = [0]

        def load(dst, src):
            _dq[_di[0] % 2].dma_start(out=dst, in_=src)
            _di[0] += 1

        def load_wih(L):
            src = w_d[L + "_wih"].rearrange("(kt p) m -> p kt m", kt=4)
            for kk in range(4):
                load(wih_sb[L][:, kk], src[:, kk])

        def load_whh(L):
            src = w_d[L + "_whh"].rearrange("(kt p) m -> p kt m", kt=2)
            for kk in range(2):
                load(whh_sb[L][:, kk], src[:, kk])
            if has_bias:
                load(biasT_sb[L], w_d[L + "_bias"][:, :])

        for rep in range(reps):
            sfx = f"r{rep}"
            # --- P1 input: x stream, edges first (both dirs consume edges);
            # on the first rep the weight/constant loads slot in between ---
            xT = big.tile([128, 4, PLOC], F8, tag="xT", name=f"xT_{sfx}")
            xsrc = xT_d.rearrange("(kt p) f -> p kt f", kt=4)
            wblocks = [(0, 3), (25, 28), (3, 8), (20, 25), (8, 14), (14, 20)]
            for bi, (lo, hi) in enumerate(wblocks):
                for kk in range(4):
                    load(xT[:, kk, lo * 224:hi * 224],
                         xsrc[:, kk, lo * 224:hi * 224])
                if rep == 0:
                    if bi == 1:
                        for L in ("vf", "vb"):
                            load_wih(L)
                            load_whh(L)
                    elif bi == 3:
                        for L in ("hf", "hb"):
                            load_wih(L)
                            load_whh(L)
                    elif bi == 5:
                        load(fcw_sb, fcw_d.rearrange("(kt p) n -> p kt n", kt=4))
                        if has_bias:
                            load(fcb_sb, fcb_d[:, :])
                        load(patchT_sb, patchT_d.rearrange("b k c -> k b c"))

            # P1 output slab, P2-read order: free = r*896 + kt*224 + (b*28+w)
            Hv = big.tile([128, T, 4, 224], F8, tag="Hv", name=f"Hv_{sfx}")
            HvS = Hv.rearrange("p r q (b w) -> p q b r w", b=BL)

            def rhs1(d, j, pos, _xT=xT):
                return _xT[:, 2 * j:2 * j + 2, pos * 224:(pos + 1) * 224]

            def hv_dst(d, pos, _HvS=HvS):
                return _HvS[:, 2 * d:2 * d + 2, :, :, pos]

            rings1 = [state.tile([128, 3, 2, 224], F8, tag=f"ring1_{d}",
                                 name=f"ring1_{d}_{sfx}") for d in range(2)]
            cs1 = [state.tile([128, 2, 224], F16, tag=f"c1_{d}",
                              name=f"c1_{d}_{sfx}") for d in range(2)]
            with tc.tile_pool(name="gates1", bufs=2, space="PSUM") as gp:
                for t in range(T):
                    _emit_pair(nc, gp, scr,
                               [wih_sb["vf"], wih_sb["vb"]],
                               [whh_sb["vf"], whh_sb["vb"]],
                               [biasT_sb["vf"], biasT_sb["vb"]], ones224,
                               rhs1, rings1, cs1, t, 1, f"1{sfx}", has_bias,
                               h16_dst=None, hv_dst=hv_dst)

            # --- P2: horizontal bi-LSTM ---
            Hh = big.tile([128, 4, PLOC], F16, tag="Hh", name=f"Hh_{sfx}")

            def rhs2(d, j, pos, _Hv=Hv):
                return _Hv[:, pos, 2 * j:2 * j + 2, :]

            def h16_dst(d, pos, _Hh=Hh):
                return _Hh[:, 2 * d:2 * d + 2, pos * 224:(pos + 1) * 224]

            rings2 = [state.tile([128, 3, 2, 224], F8, tag=f"ring2_{d}",
                                 name=f"ring2_{d}_{sfx}") for d in range(2)]
            cs2 = [state.tile([128, 2, 224], F16, tag=f"c2_{d}",
                              name=f"c2_{d}_{sfx}") for d in range(2)]
            with tc.tile_pool(name="gates2", bufs=2, space="PSUM") as gp:
                for t in range(T):
                    _emit_pair(nc, gp, scr,
                               [wih_sb["hf"], wih_sb["hb"]],
                               [whh_sb["hf"], whh_sb["hb"]],
                               [biasT_sb["hf"], biasT_sb["hb"]], ones224,
                               rhs2, rings2, cs2, t, 2, f"2{sfx}", has_bias,
                               h16_dst=h16_dst, hv_dst=None)

            # --- P3: fc + softmax + transpose + einsum ---
            KT = big.tile([100, PLOC], F16, tag="KT", name=f"KT_{sfx}")
            # GPSIMD cannot touch PSUM; alternate the PSUM->SBUF copies
            # between DVE and Act (Act has slack once the LSTMs are done)
            cps = [nc.vector, nc.scalar]
            dmas = [nc.sync]
            ci = 0
            with tc.tile_pool(name="p3ps", bufs=2, space="PSUM") as pps:

                def fc_tile(half, hr):
                    nonlocal ci
                    off = hr * 224 + half * 112
                    Lp = pps.tile([112, 100], F32, tag="L", bufs=3,
                                  name=f"L_{hr}_{half}_{sfx}")
                    for kk in range(4):
                        nc.tensor.matmul(Lp, lhsT=Hh[:, kk, off:off + 112],
                                         rhs=fcw_sb[:, kk, :],
                                         start=(kk == 0),
                                         stop=(not has_bias and kk == 3))
                    if has_bias:
                        nc.tensor.matmul(Lp, lhsT=ones112, rhs=fcb_sb,
                                         start=False, stop=True)
                    E = scr.tile([112, 100], F16, tag="E", bufs=3,
                                 name=f"E_{hr}_{half}_{sfx}")
                    Zs = scr.tile([112, 1], F32, tag="Z", bufs=3,
                                  name=f"Z_{hr}_{half}_{sfx}")
                    # accum_out would add a 187ns accumulator-read on the
                    # Act engine; the free-dim sum on DVE keeps the whole
                    # normalize chain on one engine
                    nc.scalar.activation(E, Lp, AF.Exp)
                    nc.vector.tensor_reduce(Zs, E, mybir.AxisListType.X,
                                            mybir.AluOpType.add)
                    rz = scr.tile([112, 1], F32, tag="rz", bufs=3,
                                  name=f"rz_{hr}_{half}_{sfx}")
                    nc.vector.reciprocal(rz, Zs)
                    Ka = scr.tile([112, 100], F16, tag="Ka", bufs=3,
                                  name=f"Ka_{hr}_{half}_{sfx}")
                    nc.vector.tensor_scalar_mul(Ka, E, rz)
                    KTp = pps.tile([100, 112], F16, tag="KTp",
                                   name=f"KTp_{hr}_{half}_{sfx}")
                    nc.tensor.transpose(KTp, Ka, ident)
                    # KT columns p = b*784 + hr*28 + w for these positions
                    dst = KT.rearrange("k (b hw) -> k b hw", b=BL)[
                        :, half * 4:(half + 1) * 4, hr * 28:(hr + 1) * 28]
                    nc.vector.tensor_copy(dst, KTp)
                    ci += 1

                def einsum_unit(b_i, ct, j2):
                    nonlocal ci
                    lhsT = patchT_sb[:, b_i, ct * 128:(ct + 1) * 128]
                    Op = pps.tile([128, 512], F32, tag="O", bufs=3,
                                  name=f"O_{b_i}_{ct}_{j2}_{sfx}")
                    nc.tensor.matmul(
                        Op[:, 0:392], lhsT=lhsT,
                        rhs=KT[:, b_i * 784 + j2 * 392:
                               b_i * 784 + (j2 + 1) * 392],
                        start=True, stop=True)
                    ob = scr.tile([128, 392], F16, tag="ob", bufs=4,
                                  name=f"ob_{b_i}_{ct}_{j2}_{sfx}")
                    eng = cps[ci % 2]
                    if eng is nc.scalar:
                        eng.copy(ob, Op[:, 0:392])
                    else:
                        eng.tensor_copy(ob, Op[:, 0:392])
                    nc.sync.dma_start(
                        out=out_d[b_i, ct * 128:(ct + 1) * 128,
                                  j2 * 392:(j2 + 1) * 392],
                        in_=ob)
                    ci += 1

                # einsum unit (b, ct, j2) needs KT rows r<14 (j2=0) or
                # r>=14 (j2=1) of its half: ready after 14 fc tiles.
                # Interleave ready units between fc tiles to keep every
                # engine busy through the softmax chain.
                pending = []
                nfc = 0
                for half in range(2):
                    for hr in range(H):
                        fc_tile(half, hr)
                        nfc += 1
                        if nfc % 14 == 0:
                            j2 = (nfc // 14 + 1) % 2
                            pending += [(b_i, ct, j2)
                                        for b_i in range(((nfc - 1) // 28) * 4,
                                                         ((nfc - 1) // 28) * 4 + 4)
                                        for ct in range(4)]
                        for _ in range(2):
                            if pending:
                                einsum_unit(*pending.pop(0))
                for u in pending:
                    einsum_unit(*u)
            if debug and rep == reps - 1:
                nc.sync.dma_start(out=dbg_hv[:, :, :, :], in_=Hv)
                nc.sync.dma_start(out=dbg_hh[:, :, :], in_=Hh)
                nc.sync.dma_start(out=dbg_kt[:, :], in_=KT)

    nc.compile()
    return nc


_NC_CACHE = {}


def _get_nc(reps=1, debug=False, has_bias=True):
    key = (reps, debug, has_bias)
    if key not in _NC_CACHE:
        _NC_CACHE[key] = _build(reps=reps, debug=debug, has_bias=has_bias)
    return _NC_CACHE[key]


def _prep_core_inputs(x, weights_np):
    """Host-side marshalling for one core. x: [BL, C, H, W] f32."""
    f8 = ml_dtypes.float8_e4m3
    m = {}
    m["xT"] = np.ascontiguousarray(
        x.transpose(1, 3, 0, 2).reshape(C, PLOC)).astype(f8)
    m["patchT"] = np.ascontiguousarray(
        x[:, :, ::3, ::3].reshape(BL, C, 100).transpose(0, 2, 1)).astype(np.float16)
    m.update(weights_np)
    return m


def _prep_weights(inputs):
    f8 = ml_dtypes.float8_e4m3
    bf = ml_dtypes.bfloat16
    w = {}
    for L in _LSTMS:
        wih = np.asarray(inputs[L + "_Wih"], np.float32)
        whh = np.asarray(inputs[L + "_Whh"], np.float32)
        bih = np.asarray(inputs[L + "_bih"], np.float32)
        bhh = np.asarray(inputs[L + "_bhh"], np.float32)
        w[L + "_wih"] = np.ascontiguousarray(wih[_PERM].T).astype(f8)
        w[L + "_whh"] = np.ascontiguousarray(whh[_PERM].T).astype(f8)
        w[L + "_bias"] = np.ascontiguousarray(
            (bih + bhh)[_PERM].reshape(1, 1024)).astype(bf)
    w["fcw"] = np.asarray(inputs["fc_W"], np.float32).astype(np.float16)
    w["fcb"] = np.asarray(inputs["fc_b"], np.float32).reshape(1, 100).astype(np.float16)
    return w


def run_cores(inputs, reps=1, debug=False):
    x = np.asarray(inputs["x"], np.float32)
    wnp = _prep_weights(inputs)
    has_bias = any(np.any(np.asarray(wnp[L + "_bias"], np.float32))
                   for L in _LSTMS) or np.any(np.asarray(wnp["fcb"], np.float32))
    nc = _get_nc(reps=reps, debug=debug, has_bias=has_bias)
    in_maps = [
        _prep_core_inputs(x[ci * BL:(ci + 1) * BL], wnp) for ci in range(N_CORES)
    ]
    res = run_bass_kernel_spmd(nc, in_maps, list(range(N_CORES)))
    return res


def kernel(**inputs) -> np.ndarray:
    res = run_cores(inputs)
    out = np.concatenate(
        [res.results[ci]["out"].reshape(BL, C, H, W) for ci in range(N_CORES)],
        axis=0)
    return out.astype(np.float32)
